# revision 16
# baseline (speedup 1.0000x reference)
"""Trainium2 Bass kernel for the Actor MLP scorer (gnn_message_passing).

Computation (see reference):
    node_e  = node_embeddings[action_nodes]          # [A, 128] gather
    feats   = [node_e | region_embeddings[action_regions] | const_tail]   # [A, 1427]
    h1..h3  = relu MLP (256 wide), logits = h3 @ W4 + b4                  # [A]
    probs   = softmax(logits) over ALL actions

Strategy (8 NeuronCores, data-parallel over actions):
  - Shard A=100000 actions as 12500/core.  Per core, actions are sorted by
    node-id bucket (< 32768 vs >= 32768) so the node-embedding gather can use
    the int16-indexed DMA-gather ucode with two base-offset views of a bf16
    copy of the table; transpose mode deposits embeddings directly in
    [dim, action] layout (no on-chip transposes).  Groups are padded to the
    static capacities C0/C1 (~7 sigma for uniform node ids); a mask input
    removes pad slots from the softmax.  Outputs are un-permuted on host.
  - Layer 1 is decomposed: feats @ W1 = node_e @ W1[:128]
        + onehot(region) @ (region_embeddings @ W1[128:256])
        + tail @ W1[256:]  (constant -> folded into the relu bias).
    The constant tail/region projections are computed on-device in a
    fp32 prologue.
  - Activations stay transposed ([feature, action]); matmuls are bf16 with
    fp32 PSUM accumulation; relu+bias evictions split across ScalarE/VectorE.
  - Softmax: per-core sum(exp(logit - 4)), one [1,1] AllReduce over the 8
    cores, then probs = exp * (1/S) on-core.
"""

import sys

for _p in ("/opt/trn_rl_repo",):
    if _p not in sys.path:
        sys.path.insert(0, _p)

import numpy as np
import ml_dtypes
from concourse import bass, bacc, mybir, tile
from concourse import bass_utils
from concourse.masks import make_identity


# ---------------------------------------------------------------- constants
N_CORES = 8
A_FULL = 100000
N_NODES = 50000
N_REGIONS = 8
D = 128
H = 256
G = 147
IN_DIM = 2 * D + N_REGIONS * D + G          # 1427
TAIL_LEN = N_REGIONS * D + G                # 1171
TAIL_KT = 10                                # ceil(1171/128)
F32 = mybir.dt.float32
BF16 = mybir.dt.bfloat16
I16 = mybir.dt.int16

A_PC = A_FULL // N_CORES                    # 12500
SPLIT = 32768                               # int16 index range boundary
C0 = 8704                                   # capacity, node id < 32768 (17*512)
C1 = 4608                                   # capacity, node id >= 32768 (9*512)
A_PAD = C0 + C1                             # 13312 = 26*512 = 104*128
N_CHUNKS = A_PAD // 128                     # 104
ATILE = 512
N_AT = A_PAD // ATILE                       # 26
GCHUNK = 512                                # idxs per dma_gather call

EXP_SHIFT = -4.0


def _gather_chunks(total):
    out, off = [], 0
    while off < total:
        n = min(GCHUNK, total - off)
        out.append((off, n))
        off += n
    return out


def build_graph():
    nc = bacc.Bacc("TRN2", target_bir_lowering=False, debug=False,
                   num_devices=N_CORES, num_swdge_queues=4)

    # ---- I/O --------------------------------------------------------------
    node_emb = nc.dram_tensor("node_emb", [N_NODES, D], BF16, kind="ExternalInput")
    w1 = nc.dram_tensor("w1", [IN_DIM, H], F32, kind="ExternalInput")
    w2 = nc.dram_tensor("w2", [H, H], F32, kind="ExternalInput")
    w3 = nc.dram_tensor("w3", [H, H], F32, kind="ExternalInput")
    w4c = nc.dram_tensor("w4c", [D, 2], F32, kind="ExternalInput")
    b1c_in = nc.dram_tensor("b1c", [D, 2], F32, kind="ExternalInput")
    b2c_in = nc.dram_tensor("b2c", [D, 2], F32, kind="ExternalInput")
    b3c_in = nc.dram_tensor("b3c", [D, 2], F32, kind="ExternalInput")
    b4_in = nc.dram_tensor("b4", [1, 1], F32, kind="ExternalInput")
    regT = nc.dram_tensor("regT", [D, N_REGIONS], F32, kind="ExternalInput")
    tailc = nc.dram_tensor("tailc", [D, TAIL_KT], F32, kind="ExternalInput")
    idx0 = nc.dram_tensor("idx0", [128, C0 // 16], I16, kind="ExternalInput")
    idx1 = nc.dram_tensor("idx1", [128, C1 // 16], I16, kind="ExternalInput")
    onehot = nc.dram_tensor("onehot", [N_REGIONS, A_PAD], BF16, kind="ExternalInput")
    mask_in = nc.dram_tensor("mask", [128, N_CHUNKS], F32, kind="ExternalInput")

    out_logits = nc.dram_tensor("out_logits", [1, A_PAD], F32, kind="ExternalOutput")
    out_probs = nc.dram_tensor("out_probs", [128, N_CHUNKS], F32, kind="ExternalOutput")

    with tile.TileContext(nc) as tc:
        with (
            tc.tile_pool(name="const", bufs=1) as cpool,
            tc.tile_pool(name="hbuf", bufs=2) as hpool,
            tc.tile_pool(name="graw", bufs=6) as gpool,
            tc.tile_pool(name="pnt", bufs=2, space="PSUM") as pnt_pool,
            tc.tile_pool(name="ph", bufs=2, space="PSUM") as ph_pool,
            tc.tile_pool(name="plg", bufs=2, space="PSUM") as plg_pool,
            tc.tile_pool(name="dram", bufs=1, space="DRAM") as dpool,
        ):
            # ---- constant loads (bf16 weights via SWDGE cast-DMA) --------
            w1a = cpool.tile([128, H], BF16, tag="w1a")
            nc.gpsimd.dma_start(out=w1a[:], in_=w1[0:D, :])
            w2t = [cpool.tile([128, H], BF16, tag=f"w2_{k}", name=f"w2_{k}")
                   for k in range(2)]
            w3t = [cpool.tile([128, H], BF16, tag=f"w3_{k}", name=f"w3_{k}")
                   for k in range(2)]
            for k in range(2):
                nc.gpsimd.dma_start(out=w2t[k][:], in_=w2[k * 128:(k + 1) * 128, :])
                nc.gpsimd.dma_start(out=w3t[k][:], in_=w3[k * 128:(k + 1) * 128, :])
            w4s = cpool.tile([128, 2], BF16, tag="w4s")
            nc.gpsimd.dma_start(out=w4s[:], in_=w4c[:])

            w1b = cpool.tile([128, H], BF16, tag="w1b")
            nc.gpsimd.dma_start(out=w1b[:], in_=w1[D:2 * D, :])
            b2s = cpool.tile([128, 2], F32, tag="b2s")
            nc.sync.dma_start(out=b2s[:], in_=b2c_in[:])
            b3s = cpool.tile([128, 2], F32, tag="b3s")
            nc.sync.dma_start(out=b3s[:], in_=b3c_in[:])
            b4s = cpool.tile([1, 1], F32, tag="b4s")
            nc.sync.dma_start(out=b4s[:], in_=b4_in[:])
            b1s = cpool.tile([128, 2], F32, tag="b1s")
            nc.sync.dma_start(out=b1s[:], in_=b1c_in[:])
            regTs = cpool.tile([128, N_REGIONS], BF16, tag="regTs")
            nc.gpsimd.dma_start(out=regTs[:], in_=regT[:])
            tails = cpool.tile([128, TAIL_KT], BF16, tag="tails")
            nc.gpsimd.dma_start(out=tails[:], in_=tailc[:])
            ohs = cpool.tile([N_REGIONS, A_PAD], BF16, tag="ohs")
            nc.sync.dma_start(out=ohs[:], in_=onehot[:])
            masks = cpool.tile([128, N_CHUNKS], F32, tag="masks")
            nc.sync.dma_start(out=masks[:], in_=mask_in[:])
            i0 = cpool.tile([128, C0 // 16], I16, tag="i0")
            nc.sync.dma_start(out=i0[:], in_=idx0[:])
            i1 = cpool.tile([128, C1 // 16], I16, tag="i1")
            nc.sync.dma_start(out=i1[:], in_=idx1[:])

            # ---- node gather (row-major) + xbar transpose ----------------
            # nts_all[d, slot] = node_emb[node_id(slot), d]  (bf16)
            nts_all = cpool.tile([128, A_PAD], BF16, tag="nts_all")
            ident = cpool.tile([128, 128], BF16, tag="ident")
            make_identity(nc, ident[:])
            gather_plan = (
                [(0, off, n, 0) for off, n in _gather_chunks(C0)]
                + [(C0, off, n, 1) for off, n in _gather_chunks(C1)])
            # process chunk pairs: two gathers, 8 PE transposes into one
            # [128, 1024] bf16 PSUM tile, one eviction
            for pi in range(0, len(gather_plan), 2):
                pair = gather_plan[pi:pi + 2]
                nt_ps = pnt_pool.tile([128, 2 * GCHUNK], BF16, space="PSUM",
                                      tag="nt_ps", name="nt_ps")
                p_used = 0
                s_base = pair[0][0] + pair[0][1]
                for gi, (zone, off, n, grp) in enumerate(pair):
                    graw = gpool.tile([128, n // 128, D], BF16, tag="graw",
                                      name="graw")
                    src = node_emb[0:SPLIT, :] if grp == 0 \
                        else node_emb[SPLIT:N_NODES, :]
                    itile = i0 if grp == 0 else i1
                    nc.gpsimd.dma_gather(
                        out_ap=graw[:],
                        in_ap=src,
                        idxs_ap=itile[:, off // 16:(off + n) // 16],
                        num_idxs=n, num_idxs_reg=n,
                        elem_size=D, transpose=False, single_packet=False,
                        queue_num=1 + (pi + gi) % 3)
                    for c in range(n // 128):
                        nc.tensor.transpose(
                            out=nt_ps[:, p_used + c * 128:p_used + (c + 1) * 128],
                            in_=graw[:, c, :], identity=ident[:])
                    p_used += n
                eng = "act" if (pi // 2) % 2 == 0 else "dve"
                if eng == "act":
                    nc.scalar.activation(
                        out=nts_all[:, s_base:s_base + p_used],
                        in_=nt_ps[:, 0:p_used],
                        func=mybir.ActivationFunctionType.Copy)
                else:
                    nc.vector.tensor_copy(
                        out=nts_all[:, s_base:s_base + p_used],
                        in_=nt_ps[:, 0:p_used])

            # ---- prologue: RP = region_emb @ W1b (fp32, [region, j]) -----
            rp_ps = plg_pool.tile([8, H], F32, space="PSUM", tag="lg")
            nc.tensor.matmul(out=rp_ps[:], lhsT=regTs[:], rhs=w1b[:],
                             start=True, stop=True)
            rps = cpool.tile([8, H], BF16, tag="rps")
            nc.vector.tensor_copy(out=rps[:], in_=rp_ps[:])

            # ---- prologue: c_tail = tail @ W1[256:] + b1 (fp32) ----------
            w1tt = [cpool.tile([128, H], BF16, tag=f"w1t_{kt}", name=f"w1t_{kt}")
                    for kt in range(TAIL_KT)]
            for kt in range(TAIL_KT):
                r0 = 2 * D + kt * 128
                r1 = min(2 * D + (kt + 1) * 128, IN_DIM)
                nc.gpsimd.dma_start(out=w1tt[kt][0:r1 - r0, :], in_=w1[r0:r1, :])
            ct_ps = plg_pool.tile([128, 2], F32, space="PSUM", tag="lg")
            for j in range(2):
                for kt in range(TAIL_KT):
                    kk = min(128, TAIL_LEN - kt * 128)
                    nc.tensor.matmul(
                        out=ct_ps[:, j:j + 1],
                        lhsT=w1tt[kt][0:kk, j * 128:(j + 1) * 128],
                        rhs=tails[0:kk, kt:kt + 1],
                        start=(kt == 0), stop=(kt == TAIL_KT - 1))
            b1cs = cpool.tile([128, 2], F32, tag="b1cs")
            nc.vector.tensor_add(out=b1cs[:], in0=ct_ps[:], in1=b1s[:])

            lrow = cpool.tile([1, A_PAD], F32, tag="lrow")

            def evict_relu(engine, dst, src, bias_ap):
                if engine == "act":
                    nc.scalar.activation(
                        out=dst, in_=src,
                        func=mybir.ActivationFunctionType.Relu, bias=bias_ap)
                else:
                    nc.vector.tensor_scalar(
                        out=dst, in0=src, scalar1=bias_ap, scalar2=0.0,
                        op0=mybir.AluOpType.add, op1=mybir.AluOpType.max)

            # ---- main loop: sweeps of up to 4 action tiles ----------------
            # Layer-major inside a sweep so consecutive matmuls share their
            # stationary operand (walrus ldw-opt then elides the reloads).
            SWEEP = 2
            t0s = list(range(0, N_AT, SWEEP))
            for t0 in t0s:
                tiles = list(range(t0, min(t0 + SWEEP, N_AT)))
                sls = [slice(t * ATILE, (t + 1) * ATILE) for t in tiles]
                nt = len(tiles)

                # h tiles hold both sweep-tiles side by side: [128, 2*ATILE]
                h1 = [hpool.tile([128, nt * ATILE], BF16, tag=f"h1_{j}",
                                 name=f"h1_{j}") for j in range(2)]
                for j in range(2):
                    hp = ph_pool.tile([128, nt * ATILE], F32, space="PSUM",
                                      tag="hps", name="hps")
                    for i in range(nt):
                        nc.tensor.matmul(out=hp[:, i * ATILE:(i + 1) * ATILE],
                                         lhsT=w1a[:, j * 128:(j + 1) * 128],
                                         rhs=nts_all[:, sls[i]],
                                         start=True, stop=False)
                    for i in range(nt):
                        nc.tensor.matmul(out=hp[:, i * ATILE:(i + 1) * ATILE],
                                         lhsT=rps[0:8, j * 128:(j + 1) * 128],
                                         rhs=ohs[0:8, sls[i]],
                                         start=False, stop=True)
                    evict_relu("act" if j == 0 else "dve",
                               h1[j][:], hp[:], b1cs[:, j:j + 1])

                # layers 2 and 3
                hin = h1
                for li, (wt, bs) in enumerate(((w2t, b2s), (w3t, b3s))):
                    hout = [hpool.tile([128, nt * ATILE], BF16,
                                       tag=f"h{li + 2}_{j}",
                                       name=f"h{li + 2}_{j}") for j in range(2)]
                    for j in range(2):
                        hp = ph_pool.tile([128, nt * ATILE], F32, space="PSUM",
                                          tag="hps", name="hps")
                        for k in range(2):
                            for i in range(nt):
                                nc.tensor.matmul(
                                    out=hp[:, i * ATILE:(i + 1) * ATILE],
                                    lhsT=wt[k][:, j * 128:(j + 1) * 128],
                                    rhs=hin[k][:, i * ATILE:(i + 1) * ATILE],
                                    start=(k == 0), stop=(k == 1))
                        evict_relu("act" if (j + li) % 2 == 0 else "dve",
                                   hout[j][:], hp[:], bs[:, j:j + 1])
                    hin = hout

                # layer 4: logits
                for i in range(nt):
                    lg = plg_pool.tile([1, ATILE], F32, space="PSUM", tag="lg")
                    for k in range(2):
                        nc.tensor.matmul(
                            out=lg[:], lhsT=w4s[:, k:k + 1],
                            rhs=hin[k][:, i * ATILE:(i + 1) * ATILE],
                            start=(k == 0), stop=(k == 1))
                    nc.vector.tensor_scalar_add(
                        out=lrow[0:1, sls[i]], in0=lg[:], scalar1=b4s[0:1, 0:1])

            # ---- store logits -------------------------------------------
            nc.sync.dma_start(out=out_logits[:], in_=lrow[:])

            # ---- softmax ------------------------------------------------
            lgT = cpool.tile([128, N_CHUNKS], F32, tag="lgT")
            nc.sync.dma_start(
                out=lgT[:],
                in_=out_logits[0:1, :].rearrange("o (p t) -> (o p) t", p=128))
            expt = cpool.tile([128, N_CHUNKS], F32, tag="expt")
            shift = cpool.tile([128, 1], F32, tag="shift")
            nc.gpsimd.memset(shift[:], EXP_SHIFT)
            nc.scalar.activation(out=expt[:], in_=lgT[:],
                                 func=mybir.ActivationFunctionType.Exp,
                                 bias=shift[:], scale=1.0)
            em = cpool.tile([128, N_CHUNKS], F32, tag="em")
            nc.vector.tensor_tensor(out=em[:], in0=expt[:], in1=masks[:],
                                    op=mybir.AluOpType.mult)
            srow = cpool.tile([128, 1], F32, tag="srow")
            nc.vector.tensor_reduce(out=srow[:], in_=em[:],
                                    axis=mybir.AxisListType.X,
                                    op=mybir.AluOpType.add)
            from concourse import bass_isa
            sall = cpool.tile([128, 1], F32, tag="sall")
            nc.gpsimd.partition_all_reduce(out_ap=sall[:], in_ap=srow[:],
                                           channels=128,
                                           reduce_op=bass_isa.ReduceOp.add)
            s_sb = cpool.tile([1, 1], F32, tag="s_sb")
            nc.vector.tensor_copy(out=s_sb[:], in_=sall[0:1, :])

            cc_in = dpool.tile([1, 1], F32, name="cc_in")
            cc_out = dpool.tile([1, 1], F32, addr_space="Shared", name="cc_out")
            nc.gpsimd.dma_start(out=cc_in[:], in_=s_sb[:])
            nc.gpsimd.collective_compute(
                "AllReduce", mybir.AluOpType.add,
                replica_groups=[list(range(N_CORES))],
                ins=[cc_in.opt()], outs=[cc_out.opt()])
            sg = cpool.tile([1, 1], F32, tag="sg")
            nc.gpsimd.dma_start(out=sg[:], in_=cc_out[:])

            sgb = cpool.tile([128, 1], F32, tag="sgb")
            nc.gpsimd.partition_broadcast(out_ap=sgb[:], in_ap=sg[:])
            rb = cpool.tile([128, 1], F32, tag="rb")
            nc.vector.reciprocal(out=rb[:], in_=sgb[:])

            probs = cpool.tile([128, N_CHUNKS], F32, tag="probs")
            nc.vector.tensor_scalar_mul(out=probs[:], in0=em[:], scalar1=rb[:])
            nc.sync.dma_start(out=out_probs[:], in_=probs[:])

    nc.compile()
    return nc


_GRAPH_CACHE = {}


def _get_graph():
    if "g" not in _GRAPH_CACHE:
        _GRAPH_CACHE["g"] = build_graph()
    return _GRAPH_CACHE["g"]


def _wrap_idx(ix):
    """int16 index layout for dma_gather: [16, N/16] column-wrapped,
    replicated 8x down the partitions."""
    w = ix.reshape(-1, 16).T
    return np.ascontiguousarray(np.tile(w, (8, 1)))


def make_in_maps(node_embeddings, region_embeddings, global_context,
                 W1, b1, W2, b2, W3, b3, W4, b4,
                 action_nodes, action_regions):
    """Host-side sharding / marshalling. Returns (in_maps, per-core metas)."""
    W1 = np.ascontiguousarray(W1, dtype=np.float32)
    W2 = np.ascontiguousarray(W2, dtype=np.float32)
    W3 = np.ascontiguousarray(W3, dtype=np.float32)
    an = np.asarray(action_nodes).astype(np.int64)
    ar = np.asarray(action_regions).astype(np.int64)
    node_bf16 = np.ascontiguousarray(
        np.asarray(node_embeddings, np.float32).astype(ml_dtypes.bfloat16))

    tail = np.concatenate([
        np.asarray(region_embeddings, np.float32).reshape(-1),
        np.asarray(global_context, np.float32).reshape(-1)])
    tail_pad = np.zeros(TAIL_KT * 128, np.float32)
    tail_pad[:TAIL_LEN] = tail
    tailc = np.ascontiguousarray(tail_pad.reshape(TAIL_KT, 128).T)

    w4c = np.ascontiguousarray(np.asarray(W4, np.float32).reshape(2, 128).T)
    b1c = np.ascontiguousarray(np.asarray(b1, np.float32).reshape(2, 128).T)
    b2c = np.ascontiguousarray(np.asarray(b2, np.float32).reshape(2, 128).T)
    b3c = np.ascontiguousarray(np.asarray(b3, np.float32).reshape(2, 128).T)
    b4m = np.asarray(b4, np.float32).reshape(1, 1)
    regTm = np.ascontiguousarray(np.asarray(region_embeddings, np.float32).T)

    in_maps, metas = [], []
    for c in range(N_CORES):
        s = c * A_PC
        nodes = an[s:s + A_PC]
        regions = ar[s:s + A_PC]
        grp = (nodes >= SPLIT).astype(np.int8)
        order = np.argsort(grp, kind="stable")      # group0 first, stable
        c0 = int((grp == 0).sum())
        c1 = A_PC - c0
        if c0 > C0 or c1 > C1:
            raise RuntimeError(
                f"core {c}: group sizes {c0}/{c1} exceed capacities {C0}/{C1}")
        sn = nodes[order]
        sr = regions[order]

        ix0 = np.zeros(C0, np.int16)
        ix0[:c0] = sn[:c0].astype(np.int16)
        ix1 = np.zeros(C1, np.int16)
        ix1[:c1] = (sn[c0:] - SPLIT).astype(np.int16)

        slots = np.concatenate([np.arange(c0), C0 + np.arange(c1)])
        oh = np.zeros((N_REGIONS, A_PAD), ml_dtypes.bfloat16)
        oh[sr, slots] = 1.0
        mask = np.zeros(A_PAD, np.float32)
        mask[slots] = 1.0

        in_maps.append({
            "node_emb": node_bf16,
            "w1": W1, "w2": W2, "w3": W3,
            "w4c": w4c, "b1c": b1c, "b2c": b2c, "b3c": b3c, "b4": b4m,
            "regT": regTm, "tailc": tailc,
            "idx0": _wrap_idx(ix0), "idx1": _wrap_idx(ix1),
            "onehot": oh, "mask": mask.reshape(128, N_CHUNKS),
        })
        metas.append((order, slots))
    return in_maps, metas


def kernel(**inputs):
    nc = _get_graph()
    in_maps, metas = make_in_maps(**inputs)
    res = bass_utils.run_bass_kernel_spmd(
        nc, in_maps, core_ids=list(range(N_CORES)))
    probs = np.empty(A_FULL, np.float32)
    logits = np.empty(A_FULL, np.float32)
    for c in range(N_CORES):
        order, slots = metas[c]
        out = res.results[c]
        lg = out["out_logits"].reshape(-1)[slots]
        pb = out["out_probs"].reshape(-1)[slots]
        logits[c * A_PC:(c + 1) * A_PC][order] = lg
        probs[c * A_PC:(c + 1) * A_PC][order] = pb
    return probs, logits
